# revision 20
# baseline (speedup 1.0000x reference)
"""8-core Trainium2 attention kernel (Bass/Tile), nn_AttentionLayer.

Reference computation (B=2, S=4096, E=512, H=8, DH=64, scale=H=8):
    q = x @ Wq ; k = x @ Wk ; v = x @ Wv        (per batch)
    per head: scores = (q_h @ k_h^T) / 8 ; P = softmax(scores)
    out_h = P @ v_h ; concat heads

Sharding (no collectives needed): core = b*4 + hp handles batch b and head
pair hp (2 heads = 128 weight columns). Each core's output slice is
independent; host concatenates.

Device-side layout tricks (host does all transposes / casts / final divide):
  - host passes xT = x[b].T (bf16), per-head-pair weight slices (bf16)
  - scores computed TRANSPOSED (S^T[j,i]) so no on-device transposes anywhere
  - softmax denominator via ones-columns in the augmented V stationary
    (layout [1 | v0 | pad63] [v1 | 1 | pad63], full-128 stationary per head)
  - exp(S^T/8) is split scalar/vector at EXP_SPLIT columns: the scalar
    engine (exact LUT exp, 0.833 ns/col) takes the larger share, the vector
    engine (Schraudolph bit-trick exp, 1.04 ns/col) the rest, so both
    engines finish together (~670ns/unit each)
  - ph1 (scores) is emitted TWO units ahead of exp/ph2: the PE queue is
    strict FIFO, so the stalling ph2 must sit behind the next units' ph1s
  - dummy warm-up matmuls during the input-DMA window ramp the PE HAM
    clock gate (1.2 -> 2.4 GHz) before the real projections start
  - device returns unnormalized O^T (64 rows) + denominator row per head;
    host divides and transposes back.
"""

import numpy as np
import ml_dtypes

B, S, E, H = 2, 4096, 512, 8
DH = 64
SCALE = 8.0
N_CORES = 8
EC = E // 128   # 4 e-chunks (contraction chunks for projections)
NJ = S // 128   # 32 j-chunks
NI = S // 512   # 8 i-tiles
NS = S // 512   # 8 s-tiles (q/k projections)
NSC = S // 128  # 32 s-chunks (v projection)

# exp is split per head: scalar (exact LUT exp) takes head0's 512 columns,
# vector (Schraudolph) head1's.  Each engine writes its OWN e-tile — a
# shared tile would create cross-engine writer-ordering edges in the tile
# tracker that lock the two engines into a serialized sem chain.

_NC_CACHE = None


def _emit(tc, bass, mybir):
    from contextlib import ExitStack
    from collections import deque

    f32 = mybir.dt.float32
    bf16 = mybir.dt.bfloat16
    i16 = mybir.dt.int16
    Exp = mybir.ActivationFunctionType.Exp
    nc = tc.nc

    # Schraudolph fast-exp constants (bf16 bit trick):
    #   bf16_bits(exp(s/8)) ~= round(s * (128*log2e/8) + (127*128 + bias))
    # ~1% rms error on the softmax output; applied to the vector share.
    SCH_C1 = 128.0 * 1.4426950408889634 / SCALE
    SCH_C2 = 127.0 * 128.0 - 5.5

    # xT arrives slice-major ([p, s-tile, c*512+s']) so each s-slice DMA is
    # one fully-contiguous 4KB-per-partition transfer (few, large packets).
    xT_t = nc.dram_tensor("xT", [128, NS, EC * 512], bf16, kind="ExternalInput")
    wq_t = nc.dram_tensor("wq", [E, 128], bf16, kind="ExternalInput")
    wk_t = nc.dram_tensor("wk", [E, 128], bf16, kind="ExternalInput")
    wv_t = nc.dram_tensor("wv", [E, 128], bf16, kind="ExternalInput")
    out_t = nc.dram_tensor("out", [130, S], f32, kind="ExternalOutput")

    with ExitStack() as ctx:
        singles = ctx.enter_context(tc.tile_pool(name="singles", bufs=1))

        # ---- load inputs. Dispatch order: wk, xT slice 0 (in 4 e-chunk
        # pieces so the first kq chunk can start as early as possible), wq,
        # wv, then slices 1-7.
        w_sb = {}
        xt_sb = singles.tile([128, NS, EC * 512], bf16, name="xt")
        for nm, t_dram in (("wk", wk_t),):
            t = singles.tile([128, EC, 128], bf16, name=f"{nm}sb")
            nc.sync.dma_start(
                out=t, in_=t_dram[:, :].rearrange("(c p) d -> p c d", p=128)
            )
            w_sb[nm] = t
        for c in range(EC):
            nc.sync.dma_start(
                out=xt_sb[:, 0, c * 512 : (c + 1) * 512],
                in_=xT_t[:, 0, c * 512 : (c + 1) * 512],
            )
        for nm, t_dram in (("wq", wq_t), ("wv", wv_t)):
            t = singles.tile([128, EC, 128], bf16, name=f"{nm}sb")
            nc.sync.dma_start(
                out=t, in_=t_dram[:, :].rearrange("(c p) d -> p c d", p=128)
            )
            w_sb[nm] = t
        for st in range(1, NS):
            nc.sync.dma_start(out=xt_sb[:, st], in_=xT_t[:, st])

        qT = singles.tile([128, S], bf16, name="qT")
        kT = singles.tile([128, S], bf16, name="kT")
        # v_aug chunk layout: [1 | v0 | pad63] [v1 | 1 | pad63] — each head's
        # ph2 stationary is a full 128 columns so LDWEIGHTS takes the fast
        # (FWL) path; the pad columns are ones, producing garbage output rows
        # 65-127 that are simply never read. head0 out rows = [den; O],
        # head1 out rows = [O; den].
        v_aug = singles.tile([128, NSC, 256], bf16, name="v_aug")
        # memset on the otherwise-idle gpsimd engine: on the vector queue
        # this 8K-column fill takes ~7us and blocks the prologue projection
        # casts (delaying the first ph1 by ~2us).
        nc.gpsimd.memset(v_aug, 1.0)
        # scratch for PE warm-up matmuls: initialized by the vector engine
        # immediately (no DMA dependency), so the PE can start ramping the
        # HAM clock gate at ~7us instead of waiting for the wk transfer.
        scratch = singles.tile([128, 640], bf16, name="scratch")
        nc.vector.memset(scratch, 0.0)

        # ---- PSUM: three 2-bank score slots (manually rotated), 2 o-banks
        sppool = ctx.enter_context(tc.tile_pool(name="sppsum", bufs=1, space="PSUM"))
        opool = ctx.enter_context(tc.tile_pool(name="opsum", bufs=1, space="PSUM"))
        e0pool = ctx.enter_context(tc.tile_pool(name="e0pool", bufs=8))
        e1pool = ctx.enter_context(tc.tile_pool(name="e1pool", bufs=8))
        osb_pool = ctx.enter_context(tc.tile_pool(name="osb", bufs=2))

        def emit_kq(nm, dst, st, slot, cast_eng="v"):
            # q/k projection s-tile: [d=128(2 heads), s] = sum_e W[e,d]*xT[e,s]
            s_sl = slice(st * 512, (st + 1) * 512)
            ps = sppool.tile([128, 512], f32, name="pp", tag=f"s{slot}")
            for c in range(EC):
                nc.tensor.matmul(
                    ps,
                    w_sb[nm][:, c, :],
                    xt_sb[:, st, c * 512 : (c + 1) * 512],
                    start=(c == 0),
                    stop=(c == EC - 1),
                )
            if cast_eng == "v":
                nc.vector.tensor_copy(dst[:, s_sl], ps)
            else:
                nc.scalar.copy(dst[:, s_sl], ps)

        def emit_vproj(jc, slot):
            # v natural [s=128, d=128] = sum_e xT[e,s] * Wv[e,d]
            st, off = divmod(jc, 4)
            ps = sppool.tile([128, 512], f32, name="vp", tag=f"s{slot}")
            pv = ps[:, 0:128]
            for c in range(EC):
                base = c * 512 + off * 128
                nc.tensor.matmul(
                    pv,
                    xt_sb[:, st, base : base + 128],
                    w_sb["wv"][:, c, :],
                    start=(c == 0),
                    stop=(c == EC - 1),
                )
            nc.vector.tensor_copy(v_aug[:, jc, 1:65], pv[:, 0:64])
            nc.vector.tensor_copy(v_aug[:, jc, 128:192], pv[:, 64:128])

        def emit_ph1(it, jc, slot):
            # S^T[j,i] = sum_d kT[d,j] * qT[d,i]; both heads packed via
            # row tiling (K=64 each) into one 2-bank psum pair; the two
            # row-group matmuls run concurrently in the PE array.
            i_sl = slice(it * 512, (it + 1) * 512)
            j_sl = slice(jc * 128, (jc + 1) * 128)
            s_pair = sppool.tile([128, 1024], f32, name="s_pair", tag=f"s{slot}")
            for h in range(2):
                d_sl = slice(h * 64, (h + 1) * 64)
                nc.tensor.matmul(
                    s_pair[:, h * 512 : (h + 1) * 512],
                    kT[d_sl, j_sl],
                    qT[d_sl, i_sl],
                    start=True,
                    stop=True,
                    tile_position=(h * 64, 0),
                )
            return s_pair

        # ---- prologue. Warm-up dummies on the scratch tile keep the PE
        # busy (and ramp the HAM clock gate) from ~7us, while the xT slices
        # stream in; then kT/qT s-tile 0, the 3-deep ph1 pipeline, and the
        # first v chunk.
        for wdx in range(6):
            dps = sppool.tile([128, 512], f32, name="warm", tag="s0")
            nc.tensor.matmul(
                dps,
                scratch[:, 0:128],
                scratch[:, 128:640],
                start=True,
                stop=True,
            )
        emit_kq("wk", kT, 0, 1)
        emit_kq("wq", qT, 0, 2)
        s_q = deque()
        s_q.append(emit_ph1(0, 0, 0))
        s_q.append(emit_ph1(0, 1, 1))
        s_q.append(emit_ph1(0, 2, 2))
        emit_vproj(0, 0)

        # ph1 is emitted THREE units ahead: exp(u)'s ph1 dependency (via the
        # global MM-completion counter) is then ~3 periods old, so the
        # steady-state period is set by the exp engines, not by the
        # ACT -> ph2 -> ph1 -> ACT dependency cycle that a 2-deep pipeline
        # creates.  All same-slot producers within an iteration (vproj, kq,
        # then ph1(u+3)) allocate tag s{u%3} in that order: each waits only
        # on the previous producer's (fast, early-queued) evacuation casts,
        # never on a future unit's exp.
        # Remaining kT s-tiles stream inside i-tile 0; qT s-tiles stream at
        # jc==16 of the previous i-tile.
        NU = NI * NJ
        o_ps = None
        for u in range(NU):
            it, jc = divmod(u, NJ)
            i_sl = slice(it * 512, (it + 1) * 512)
            if jc == 0:
                o_ps = [
                    opool.tile([128, 512], f32, name=f"o{h}", tag=f"o{h}")
                    for h in range(2)
                ]
            if it == 0 and 1 <= u + 1 < NJ:
                emit_vproj(u + 1, u % 3)
            if it == 0 and jc % 4 == 1 and jc < 29:
                emit_kq("wk", kT, (jc + 3) // 4, u % 3, cast_eng="s")
            if jc == 16 and it + 1 < NI:
                emit_kq("wq", qT, it + 1, u % 3, cast_eng="v")
            if u + 3 < NU:
                it3, jc3 = divmod(u + 3, NJ)
                s_q.append(emit_ph1(it3, jc3, (u + 3) % 3))
            s_cur = s_q.popleft()
            # exp(S^T/8): scalar engine (exact LUT exp) takes head0's 512
            # cols, vector engine (Schraudolph) head1's — into separate
            # single-writer tiles so the engines never serialize on shared
            # bookkeeping.
            e_h0 = e0pool.tile([128, 512], bf16, name="e_h0", tag="e_h0")
            e_h1 = e1pool.tile([128, 512], bf16, name="e_h1", tag="e_h1")
            nc.scalar.activation(
                e_h0, s_cur[:, 0:512], Exp, scale=1.0 / SCALE
            )
            nc.vector.tensor_scalar(
                e_h1.bitcast(i16),
                s_cur[:, 512:1024],
                SCH_C1,
                SCH_C2,
                op0=mybir.AluOpType.mult,
                op1=mybir.AluOpType.add,
            )
            # O^T (+ denominator row) accumulated over j; h0 rows=[den;O],
            # h1 rows=[O;den], rows 65-127 garbage (pad columns)
            for h, e_h in ((0, e_h0), (1, e_h1)):
                nc.tensor.matmul(
                    o_ps[h],
                    v_aug[:, jc, h * 128 : (h + 1) * 128],
                    e_h,
                    start=(jc == 0),
                    stop=(jc == NJ - 1),
                )
            if jc == NJ - 1:
                for h in range(2):
                    o_sb = osb_pool.tile(
                        [65, 512], f32, name=f"osb{h}", tag=f"osb{h}"
                    )
                    # split the two evacuation copies across engines so the
                    # next i-tile's o-bank WAR stall is halved
                    if h == 0:
                        nc.vector.tensor_copy(o_sb, o_ps[h][0:65])
                    else:
                        nc.scalar.copy(o_sb, o_ps[h][0:65])
                    nc.sync.dma_start(
                        out=out_t[h * 65 : (h + 1) * 65, i_sl], in_=o_sb
                    )


def build_nc():
    import concourse.bass as bass
    import concourse.mybir as mybir
    import concourse.tile as tile
    from concourse import bacc

    nc = bacc.Bacc(
        "TRN2", target_bir_lowering=False, debug=False, num_devices=N_CORES
    )
    with tile.TileContext(nc) as tc:
        _emit(tc, bass, mybir)
    nc.compile()
    return nc


def _get_nc():
    global _NC_CACHE
    if _NC_CACHE is None:
        _NC_CACHE = build_nc()
    return _NC_CACHE


def make_in_maps(attention_input, Wq, Wk, Wv):
    bf16 = ml_dtypes.bfloat16
    x = np.asarray(attention_input, dtype=np.float32)
    Wq = np.asarray(Wq, dtype=np.float32)
    Wk = np.asarray(Wk, dtype=np.float32)
    Wv = np.asarray(Wv, dtype=np.float32)
    in_maps = []
    for core in range(N_CORES):
        b, hp = divmod(core, 4)
        cols = slice(hp * 128, (hp + 1) * 128)
        # xT slice-major: [p, s-tile, c*512+s'] (see _emit)
        xT4 = (
            x[b].T.reshape(EC, 128, NS, 512)
            .transpose(1, 2, 0, 3)
            .reshape(128, NS, EC * 512)
        )
        in_maps.append(
            {
                "xT": np.ascontiguousarray(xT4).astype(bf16),
                "wq": np.ascontiguousarray(Wq[:, cols]).astype(bf16),
                "wk": np.ascontiguousarray(Wk[:, cols]).astype(bf16),
                "wv": np.ascontiguousarray(Wv[:, cols]).astype(bf16),
            }
        )
    return in_maps


def assemble_output(core_outs):
    """core_outs: list of 8 arrays [130, S] f32 -> full [B, S, E] f32.

    Row layout per core: rows 0-64 = head0 [denominator; O^T], rows
    65-129 = head1 [O^T; denominator].
    """
    out = np.empty((B, S, E), np.float32)
    for core in range(N_CORES):
        b, hp = divmod(core, 4)
        o = np.asarray(core_outs[core], dtype=np.float32)
        onrm0 = o[1:65] / o[0:1]
        onrm1 = o[65:129] / o[129:130]
        out[b, :, hp * 128 : hp * 128 + 64] = onrm0.T
        out[b, :, hp * 128 + 64 : hp * 128 + 128] = onrm1.T
    return out


def kernel(attention_input, Wq, Wk, Wv, _trace=False, **trace_kwargs):
    from concourse.bass_utils import run_bass_kernel_spmd

    nc = _get_nc()
    in_maps = make_in_maps(attention_input, Wq, Wk, Wv)
    res = run_bass_kernel_spmd(
        nc, in_maps, core_ids=list(range(N_CORES)), trace=_trace, **trace_kwargs
    )
    out = assemble_output([r["out"] for r in res.results])
    if _trace:
        kernel.last_results = res
    return out


# revision 22
# speedup vs baseline: 1.1895x; 1.1895x over previous
"""8-core Trainium2 attention kernel (Bass/Tile), nn_AttentionLayer.

Reference computation (B=2, S=4096, E=512, H=8, DH=64, scale=H=8):
    q = x @ Wq ; k = x @ Wk ; v = x @ Wv        (per batch)
    per head: scores = (q_h @ k_h^T) / 8 ; P = softmax(scores)
    out_h = P @ v_h ; concat heads

Sharding (no collectives needed): core = b*4 + hp handles batch b and head
pair hp (2 heads = 128 weight columns). Each core's output slice is
independent; host concatenates.

Device-side layout tricks (host does all transposes / casts / final divide):
  - host passes xT = x[b].T (bf16), per-head-pair weight slices (bf16)
  - scores computed TRANSPOSED (S^T[j,i]) so no on-device transposes anywhere
  - softmax denominator via ones-columns in the augmented V stationary
    (layout [1 | v0 | pad63] [v1 | 1 | pad63], full-128 stationary per head)
  - exp(S^T/8) is split scalar/vector at EXP_SPLIT columns: the scalar
    engine (exact LUT exp, 0.833 ns/col) takes the larger share, the vector
    engine (Schraudolph bit-trick exp, 1.04 ns/col) the rest, so both
    engines finish together (~670ns/unit each)
  - ph1 (scores) is emitted TWO units ahead of exp/ph2: the PE queue is
    strict FIFO, so the stalling ph2 must sit behind the next units' ph1s
  - dummy warm-up matmuls during the input-DMA window ramp the PE HAM
    clock gate (1.2 -> 2.4 GHz) before the real projections start
  - device returns unnormalized O^T (64 rows) + denominator row per head;
    host divides and transposes back.
"""

import numpy as np
import ml_dtypes

B, S, E, H = 2, 4096, 512, 8
DH = 64
SCALE = 8.0
N_CORES = 8
EC = E // 128   # 4 e-chunks (contraction chunks for projections)
NJ = S // 128   # 32 j-chunks
NI = S // 512   # 8 i-tiles
NS = S // 512   # 8 s-tiles (q/k projections)
NSC = S // 128  # 32 s-chunks (v projection)

# exp is split per head: scalar (exact LUT exp) takes head0's 512 columns,
# vector (Schraudolph) head1's.  Each engine writes its OWN e-tile — a
# shared tile would create cross-engine writer-ordering edges in the tile
# tracker that lock the two engines into a serialized sem chain.

_NC_CACHE = None


def _emit(tc, bass, mybir):
    from contextlib import ExitStack
    from collections import deque

    f32 = mybir.dt.float32
    bf16 = mybir.dt.bfloat16
    i16 = mybir.dt.int16
    Exp = mybir.ActivationFunctionType.Exp
    nc = tc.nc

    # Schraudolph fast-exp constants (bf16 bit trick):
    #   bf16_bits(exp(s/8)) ~= round(s * (128*log2e/8) + (127*128 + bias))
    # ~1% rms error on the softmax output; applied to the vector share.
    SCH_C1 = 128.0 * 1.4426950408889634 / SCALE
    SCH_C2 = 127.0 * 128.0 - 5.5

    # xT arrives slice-major ([p, s-tile, c*512+s']) so each s-slice DMA is
    # one fully-contiguous 4KB-per-partition transfer (few, large packets).
    xT_t = nc.dram_tensor("xT", [128, NS, EC * 512], bf16, kind="ExternalInput")
    wq_t = nc.dram_tensor("wq", [E, 128], bf16, kind="ExternalInput")
    wk_t = nc.dram_tensor("wk", [E, 128], bf16, kind="ExternalInput")
    wv_t = nc.dram_tensor("wv", [E, 128], bf16, kind="ExternalInput")
    out_t = nc.dram_tensor("out", [130, S], f32, kind="ExternalOutput")

    with ExitStack() as ctx:
        singles = ctx.enter_context(tc.tile_pool(name="singles", bufs=1))

        # ---- load inputs. Dispatch order: wk, xT slice 0 (in 4 e-chunk
        # pieces so the first kq chunk can start as early as possible), wq,
        # wv, then slices 1-7.
        w_sb = {}
        xt_sb = singles.tile([128, NS, EC * 512], bf16, name="xt")
        for nm, t_dram in (("wk", wk_t),):
            t = singles.tile([128, EC, 128], bf16, name=f"{nm}sb")
            nc.sync.dma_start(
                out=t, in_=t_dram[:, :].rearrange("(c p) d -> p c d", p=128)
            )
            w_sb[nm] = t
        for c in range(EC):
            nc.sync.dma_start(
                out=xt_sb[:, 0, c * 512 : (c + 1) * 512],
                in_=xT_t[:, 0, c * 512 : (c + 1) * 512],
            )
        for nm, t_dram in (("wq", wq_t), ("wv", wv_t)):
            t = singles.tile([128, EC, 128], bf16, name=f"{nm}sb")
            nc.sync.dma_start(
                out=t, in_=t_dram[:, :].rearrange("(c p) d -> p c d", p=128)
            )
            w_sb[nm] = t
        for st in range(1, NS):
            nc.sync.dma_start(out=xt_sb[:, st], in_=xT_t[:, st])

        qT = singles.tile([128, S], bf16, name="qT")
        kT = singles.tile([128, S], bf16, name="kT")
        # v_aug chunk layout: [1 | v0 | pad63] [v1 | 1 | pad63] — each head's
        # ph2 stationary is a full 128 columns so LDWEIGHTS takes the fast
        # (FWL) path; the pad columns are ones, producing garbage output rows
        # 65-127 that are simply never read. head0 out rows = [den; O],
        # head1 out rows = [O; den].
        v_aug = singles.tile([128, NSC, 256], bf16, name="v_aug")
        nc.vector.memset(v_aug, 1.0)

        # ---- PSUM: three 2-bank score slots (manually rotated), 2 o-banks
        sppool = ctx.enter_context(tc.tile_pool(name="sppsum", bufs=1, space="PSUM"))
        opool = ctx.enter_context(tc.tile_pool(name="opsum", bufs=1, space="PSUM"))
        e0pool = ctx.enter_context(tc.tile_pool(name="e0pool", bufs=8))
        e1pool = ctx.enter_context(tc.tile_pool(name="e1pool", bufs=8))
        osb_pool = ctx.enter_context(tc.tile_pool(name="osb", bufs=2))

        def emit_kq(nm, dst, st, slot, cast_eng="v"):
            # q/k projection s-tile: [d=128(2 heads), s] = sum_e W[e,d]*xT[e,s]
            s_sl = slice(st * 512, (st + 1) * 512)
            ps = sppool.tile([128, 512], f32, name="pp", tag=f"s{slot}")
            for c in range(EC):
                nc.tensor.matmul(
                    ps,
                    w_sb[nm][:, c, :],
                    xt_sb[:, st, c * 512 : (c + 1) * 512],
                    start=(c == 0),
                    stop=(c == EC - 1),
                )
            if cast_eng == "v":
                nc.vector.tensor_copy(dst[:, s_sl], ps)
            else:
                nc.scalar.copy(dst[:, s_sl], ps)

        def emit_vproj(jc, slot):
            # v natural [s=128, d=128] = sum_e xT[e,s] * Wv[e,d]
            st, off = divmod(jc, 4)
            ps = sppool.tile([128, 512], f32, name="vp", tag=f"s{slot}")
            pv = ps[:, 0:128]
            for c in range(EC):
                base = c * 512 + off * 128
                nc.tensor.matmul(
                    pv,
                    xt_sb[:, st, base : base + 128],
                    w_sb["wv"][:, c, :],
                    start=(c == 0),
                    stop=(c == EC - 1),
                )
            nc.vector.tensor_copy(v_aug[:, jc, 1:65], pv[:, 0:64])
            nc.vector.tensor_copy(v_aug[:, jc, 128:192], pv[:, 64:128])

        def emit_ph1(it, jc, slot):
            # S^T[j,i] = sum_d kT[d,j] * qT[d,i]; both heads packed via
            # row tiling (K=64 each) into one 2-bank psum pair; the two
            # row-group matmuls run concurrently in the PE array.
            i_sl = slice(it * 512, (it + 1) * 512)
            j_sl = slice(jc * 128, (jc + 1) * 128)
            s_pair = sppool.tile([128, 1024], f32, name="s_pair", tag=f"s{slot}")
            for h in range(2):
                d_sl = slice(h * 64, (h + 1) * 64)
                nc.tensor.matmul(
                    s_pair[:, h * 512 : (h + 1) * 512],
                    kT[d_sl, j_sl],
                    qT[d_sl, i_sl],
                    start=True,
                    stop=True,
                    tile_position=(h * 64, 0),
                )
            return s_pair

        # ---- prologue. Warm-up dummies keep the PE busy (and ramp the HAM
        # clock gate) while the xT slices stream in; then kT/qT s-tile 0,
        # the 3-deep ph1 pipeline, and the first v chunk.
        for wdx in range(3):
            dps = sppool.tile([128, 512], f32, name="warm", tag="s0")
            nc.tensor.matmul(
                dps,
                w_sb["wk"][:, wdx % EC, :],
                w_sb["wk"][:, :, :].rearrange("p c d -> p (c d)")[:, 0:512],
                start=True,
                stop=True,
            )
        emit_kq("wk", kT, 0, 1)
        emit_kq("wq", qT, 0, 2)
        s_q = deque()
        s_q.append(emit_ph1(0, 0, 0))
        s_q.append(emit_ph1(0, 1, 1))
        s_q.append(emit_ph1(0, 2, 2))
        emit_vproj(0, 0)

        # ph1 is emitted THREE units ahead: exp(u)'s ph1 dependency (via the
        # global MM-completion counter) is then ~3 periods old, so the
        # steady-state period is set by the exp engines, not by the
        # ACT -> ph2 -> ph1 -> ACT dependency cycle that a 2-deep pipeline
        # creates.  All same-slot producers within an iteration (vproj, kq,
        # then ph1(u+3)) allocate tag s{u%3} in that order: each waits only
        # on the previous producer's (fast, early-queued) evacuation casts,
        # never on a future unit's exp.
        # Remaining kT s-tiles stream inside i-tile 0; qT s-tiles stream at
        # jc==16 of the previous i-tile.
        NU = NI * NJ
        o_ps = None
        for u in range(NU):
            it, jc = divmod(u, NJ)
            i_sl = slice(it * 512, (it + 1) * 512)
            if jc == 0:
                o_ps = [
                    opool.tile([128, 512], f32, name=f"o{h}", tag=f"o{h}")
                    for h in range(2)
                ]
            if it == 0 and 1 <= u + 1 < NJ:
                emit_vproj(u + 1, u % 3)
            if it == 0 and jc % 4 == 1 and jc < 29:
                emit_kq("wk", kT, (jc + 3) // 4, u % 3, cast_eng="s")
            if jc == 16 and it + 1 < NI:
                emit_kq("wq", qT, it + 1, u % 3, cast_eng="v")
            if u + 3 < NU:
                it3, jc3 = divmod(u + 3, NJ)
                s_q.append(emit_ph1(it3, jc3, (u + 3) % 3))
            s_cur = s_q.popleft()
            # exp(S^T/8): scalar engine (exact LUT exp) takes head0's 512
            # cols, vector engine (Schraudolph) head1's — into separate
            # single-writer tiles so the engines never serialize on shared
            # bookkeeping.
            e_h0 = e0pool.tile([128, 512], bf16, name="e_h0", tag="e_h0")
            e_h1 = e1pool.tile([128, 512], bf16, name="e_h1", tag="e_h1")
            nc.scalar.activation(
                e_h0, s_cur[:, 0:512], Exp, scale=1.0 / SCALE
            )
            nc.vector.tensor_scalar(
                e_h1.bitcast(i16),
                s_cur[:, 512:1024],
                SCH_C1,
                SCH_C2,
                op0=mybir.AluOpType.mult,
                op1=mybir.AluOpType.add,
            )
            # O^T (+ denominator row) accumulated over j; h0 rows=[den;O],
            # h1 rows=[O;den], rows 65-127 garbage (pad columns)
            for h, e_h in ((0, e_h0), (1, e_h1)):
                nc.tensor.matmul(
                    o_ps[h],
                    v_aug[:, jc, h * 128 : (h + 1) * 128],
                    e_h,
                    start=(jc == 0),
                    stop=(jc == NJ - 1),
                )
            if jc == NJ - 1:
                for h in range(2):
                    o_sb = osb_pool.tile(
                        [65, 512], f32, name=f"osb{h}", tag=f"osb{h}"
                    )
                    # split the two evacuation copies across engines so the
                    # next i-tile's o-bank WAR stall is halved
                    if h == 0:
                        nc.vector.tensor_copy(o_sb, o_ps[h][0:65])
                    else:
                        nc.scalar.copy(o_sb, o_ps[h][0:65])
                    nc.sync.dma_start(
                        out=out_t[h * 65 : (h + 1) * 65, i_sl], in_=o_sb
                    )


def build_nc():
    import concourse.bass as bass
    import concourse.mybir as mybir
    import concourse.tile as tile
    from concourse import bacc

    nc = bacc.Bacc(
        "TRN2", target_bir_lowering=False, debug=False, num_devices=N_CORES
    )
    with tile.TileContext(nc) as tc:
        _emit(tc, bass, mybir)
    nc.compile()
    return nc


def _get_nc():
    global _NC_CACHE
    if _NC_CACHE is None:
        _NC_CACHE = build_nc()
    return _NC_CACHE


def make_in_maps(attention_input, Wq, Wk, Wv):
    bf16 = ml_dtypes.bfloat16
    x = np.asarray(attention_input, dtype=np.float32)
    Wq = np.asarray(Wq, dtype=np.float32)
    Wk = np.asarray(Wk, dtype=np.float32)
    Wv = np.asarray(Wv, dtype=np.float32)
    in_maps = []
    for core in range(N_CORES):
        b, hp = divmod(core, 4)
        cols = slice(hp * 128, (hp + 1) * 128)
        # xT slice-major: [p, s-tile, c*512+s'] (see _emit)
        xT4 = (
            x[b].T.reshape(EC, 128, NS, 512)
            .transpose(1, 2, 0, 3)
            .reshape(128, NS, EC * 512)
        )
        in_maps.append(
            {
                "xT": np.ascontiguousarray(xT4).astype(bf16),
                "wq": np.ascontiguousarray(Wq[:, cols]).astype(bf16),
                "wk": np.ascontiguousarray(Wk[:, cols]).astype(bf16),
                "wv": np.ascontiguousarray(Wv[:, cols]).astype(bf16),
            }
        )
    return in_maps


def assemble_output(core_outs):
    """core_outs: list of 8 arrays [130, S] f32 -> full [B, S, E] f32.

    Row layout per core: rows 0-64 = head0 [denominator; O^T], rows
    65-129 = head1 [O^T; denominator].
    """
    out = np.empty((B, S, E), np.float32)
    for core in range(N_CORES):
        b, hp = divmod(core, 4)
        o = np.asarray(core_outs[core], dtype=np.float32)
        onrm0 = o[1:65] / o[0:1]
        onrm1 = o[65:129] / o[129:130]
        out[b, :, hp * 128 : hp * 128 + 64] = onrm0.T
        out[b, :, hp * 128 + 64 : hp * 128 + 128] = onrm1.T
    return out


def kernel(attention_input, Wq, Wk, Wv, _trace=False, **trace_kwargs):
    from concourse.bass_utils import run_bass_kernel_spmd

    nc = _get_nc()
    in_maps = make_in_maps(attention_input, Wq, Wk, Wv)
    res = run_bass_kernel_spmd(
        nc, in_maps, core_ids=list(range(N_CORES)), trace=_trace, **trace_kwargs
    )
    out = assemble_output([r["out"] for r in res.results])
    if _trace:
        kernel.last_results = res
    return out
